# revision 27
# baseline (speedup 1.0000x reference)
"""Causal dot-product attention for Trainium2 (Bass/Tile), 8-core SPMD. v3.

Problem: B=32, T=2048, D=64 fp32.  reference:
    O = softmax(mask(Q K^T / sqrt(D))) V      (causal mask, per batch)

Sharding: pure batch parallelism - 4 batches per NeuronCore, no collectives.
~82.4 us HW (from the 94.3 us v2 baseline).

v3 design (vs v2):
  * S^T matmuls use fp8e4 + DoubleRow with error-compensated operands:
    the 2x128-row contraction budget holds qh*kh + ql*kh + qh*kl (3
    terms x 64 d-rows) plus two bias rows (128*112 + 128*4 = 14848,
    each factor e4m3-exact) = 194 rows in [128 partitions, 2 k-tiles]
    (pad to 128 partitions - smaller matmuls wedge the device).  Score
    error ~0.003; 2.3e-3 end-to-end rel err on HW.
    NOTE: on HW a DoubleRow matmul still streams 1 out-col/cycle (the
    cost model's 0.5 does not materialize; byte-interleaved ifmap is
    WORSE), so fp8-DR only buys contraction depth, not rate.  The PE
    floor here is (causal column count) x 1 cyc = ~58 us/core; this
    kernel runs the PE at ~97% occupancy against that floor.
  * All pairs carry the +14848 bias; the ACT exp path compensates via
    the activation bias argument (-14848*ln2/1024).
  * Fully-masked 128-col blocks are trimmed from S^T/exp/PV (chunk B of
    the leading pair covers queries 128:512; of the diagonal pair,
    384:512) - all masks reduce to the single tri0 triangle.
  * exp split: odd non-diagonal pairs on the DVE custom fp16-bits op,
    the rest on ACT (measured-balanced); masks + epilogue casts on DVE.
  * PV stays fp16 (fp8 pexp fails the error budget: e4m3 lacks range,
    e5m2 lacks mantissa - 0.10-0.14 rel err in simulation).
  * Global cross-tile PV software pipeline (depth 2): the next tile's
    S^T fills the PE while the previous tile's last exps drain.
  * Head: batch-0's first blocks ride the Activation HWDGE queue ahead
    of the sync preamble; remaining b0 loads are split/ordered by
    consumption.  Tail: non-final output DMAs ride the Pool SW-DGE
    (prompt triggers), the final tile's copies+DMAs ride ACT, and the
    final tile's two exps run on different engines.
"""

import os

os.environ.setdefault("NEURON_RT_RESET_CORES", "1")

import numpy as np
import ml_dtypes

import concourse.bacc as bacc
import concourse.mybir as mybir
import concourse.tile as tile
from concourse.bass_utils import run_bass_kernel_spmd

B, T, D = 32, 2048, 64
NCORES = 8
BL = B // NCORES            # batches per core
P = 128                     # partitions / key-chunk size
NCH = T // P                # key chunks per batch (16)
QW = 512                    # query-tile width
NQT = T // QW               # query tiles per batch (4)

NR = 128                    # contraction partitions for the fp8 DR matmul
                            # (194 live rows + 62 zero pad; <128-partition
                            # matmuls are known-problematic on HW)
LOG2E = float(np.log2(np.e))
SQK = float(np.sqrt(128.0 * LOG2E))     # q/k prescale; SQK^2 = 128*log2e
XBIAS = 14848.0                         # (15-1)*1024 + 512
ACT_SCALE = float(np.log(2.0) / 1024.0) # exp(x*ln2/1024) = 2^(x/1024)
ACT_BIAS = float(-XBIAS * np.log(2.0) / 1024.0)

# custom DVE exp2 constants (fp16 bits domain, 1024-grid)
MAGIC = float(1.5 * 2**33)
ACONST = 423.935717
BCONST = 0.995047887        # linear (per rs)
CCONST = 3.35938581e-4      # quadratic (per rs^2)

F32 = mybir.dt.float32
F16 = mybir.dt.float16
U16 = mybir.dt.uint16
U8 = mybir.dt.uint8
F8E4 = mybir.dt.float8e4
DR = mybir.MatmulPerfMode.DoubleRow

PREWARM_N = int(os.environ.get("ATTN_PREWARM_N", "12"))


def _register_exp2_op():
    """Register the EXP2_FP16_BITS_ANT custom DVE op (idempotent)."""
    from concourse.dve_ops import (
        DveOp,
        OPS,
        _SUB_OPCODE_FOR_NAME,
        CUSTOM_DVE_SPECS,
    )
    from concourse.dve_spec import (
        Spec,
        Src0,
        C0,
        C1,
        C2,
        C3,
        lower,
        _spill_c3_to_src1,
        _has_src1,
    )
    from concourse.dve_uop import DveOpSpec

    name = "EXP2_FP16_BITS_ANT"
    if name in _SUB_OPCODE_FOR_NAME:
        return next(op for op in OPS if op.name == name)

    def _ref(in0, in1, s0, s1, imm2):
        X = in0.astype(np.float32)
        u = (X + np.float32(s0)).astype(np.float32)
        v = (u - np.float32(s0)).astype(np.float32)
        rs = (X - v).astype(np.float32)
        G = ((rs * np.float32(s1) + np.float32(imm2)) * rs).astype(np.float32)
        c3 = np.asarray(in1, np.float32).reshape(-1, 1)
        return ((G + v) + c3).astype(np.float32)

    u = Src0 + C0
    v = u - C0
    rs = Src0 - v
    G = (rs * C1 + C2) * rs
    body = _spill_c3_to_src1((G + v) + C3)
    spec = Spec(body=body, reference=_ref)
    row = max(_SUB_OPCODE_FOR_NAME.values()) + 1
    _SUB_OPCODE_FOR_NAME[name] = row
    uops = lower(spec, ver="v3")
    sha = DveOpSpec(name=name, opcode=row, uops=uops, rd1_en=_has_src1(spec)).sha(
        "v3"
    )
    op = DveOp(name, spec, subdim=False, uops_sha={"v3": sha})
    OPS.append(op)
    CUSTOM_DVE_SPECS[name] = spec
    return op


def build_nc():
    from contextlib import ExitStack

    exp2 = _register_exp2_op()
    nc = bacc.Bacc()
    # host-prepped inputs:
    #   qdr: fp8e4 bytes [NR, NQT, 2, QW] per batch - DoubleRow moving
    #        operand; row (p, j) holds the q-side factor of contraction row
    #        j*128+p (see prep_inputs); per-qtile blocks are contiguous
    #   kdr: matching k-side stationary operand [NR, NCH, 2, P]
    #   v:   V with ones column, fp16 [T, 65]
    qdr_d = nc.dram_tensor("qdr", [BL, NR, NQT, 2, QW], U8, kind="ExternalInput")
    kdr_d = nc.dram_tensor("kdr", [BL, NR, NCH, 2, P], U8, kind="ExternalInput")
    v_d = nc.dram_tensor("v", [BL, T, D + 1], F16, kind="ExternalInput")
    o_d = nc.dram_tensor("o", [BL, NQT, D + 1, QW], F16, kind="ExternalOutput")

    with tile.TileContext(nc) as tc, ExitStack() as ctx:
        singles = ctx.enter_context(tc.tile_pool(name="singles", bufs=1))
        wpool = ctx.enter_context(tc.tile_pool(name="wts", bufs=4))
        pepool = ctx.enter_context(tc.tile_pool(name="pexp", bufs=8))
        osb_pool = ctx.enter_context(tc.tile_pool(name="osb", bufs=3))
        st_ps = ctx.enter_context(tc.tile_pool(name="stps", bufs=3, space="PSUM"))
        ot_ps = ctx.enter_context(tc.tile_pool(name="otps", bufs=2, space="PSUM"))

        # critical first input blocks on the Activation HWDGE queue: ACT's
        # sequencer reaches these long before the sync engine finishes its
        # preamble, so batch 0 tile 0's data lands several us earlier.
        q0t0 = wpool.tile([NR, 2, QW], U8, tag="q0_0", name="q0_0")
        nc.scalar.dma_start(out=q0t0, in_=qdr_d[0, :, 0])
        v_r = v_d.rearrange("b (c p) d -> b p c d", p=P)
        k0t0 = wpool.tile([NR, 4, 2, P], U8, tag="k0_0", name="k0_0")
        nc.scalar.dma_start(out=k0t0, in_=kdr_d[0, :, 0:4])
        v0t0 = wpool.tile([P, 4, D + 1], F16, tag="v0_0", name="v0_0")
        nc.scalar.dma_start(out=v0t0, in_=v_r[0, :, 0:4])

        if PREWARM_N:
            # fp16 matmul burst on dummy data during the initial input-DMA
            # stall: opens the HAM clock gate / pstate before the real work.
            wsrc = singles.tile([P, QW], F16)
            nc.gpsimd.memset(wsrc, 0.5)
            wps = ot_ps.tile([P, QW], F32, tag="ot", name="warm")
            for _ in range(PREWARM_N):
                nc.tensor.matmul(
                    out=wps, lhsT=wsrc[:, 0:P], rhs=wsrc, start=True, stop=True
                )

        # precomputed 0/1 causal masks (fp16), applied by DVE multiplies:
        #   tri0: keep where f >= p      (the diagonal 128-triangle)
        #   msk1: keep where f >= 128+p  (one full masked chunk + triangle)
        tri0f = singles.tile([P, P], F32)
        nc.vector.memset(tri0f, 1.0)
        nc.gpsimd.affine_select(
            out=tri0f, in_=tri0f, compare_op=mybir.AluOpType.is_ge, fill=0.0,
            base=0, channel_multiplier=-1, pattern=[[1, P]],
        )
        tri0 = singles.tile([P, P], F16)
        nc.vector.tensor_copy(out=tri0, in_=tri0f)
        c3t = singles.tile([P, 1], F32)
        nc.vector.memset(c3t, ACONST)
        abias = singles.tile([P, 1], F32)
        nc.vector.memset(abias, ACT_BIAS)

        # preload the exp table set during the head DMA stall
        dummy = singles.tile([P, 8], F32)
        nc.vector.memset(dummy, 0.0)
        dummo = singles.tile([P, 8], F16)
        nc.scalar.activation(
            out=dummo, in_=dummy, func=mybir.ActivationFunctionType.Exp,
            scale=ACT_SCALE,
        )

        def load_batch(b):
            if b == 0:
                # consumption order: tile1 needs qdr1 + chunks 4..7 before
                # tile2's qdr2 + chunks 8..15
                qdrs = [q0t0]
                kviews = [k0t0[:, c] for c in range(4)]
                vviews = [v0t0[:, c] for c in range(4)]

                def q_load(i):
                    qt = wpool.tile([NR, 2, QW], U8, tag=f"q0_{i}", name=f"q0_{i}")
                    nc.sync.dma_start(out=qt, in_=qdr_d[b, :, i])
                    qdrs.append(qt)

                def kv_load(lo, hi, eng=None):
                    eng = eng or nc.sync
                    n = hi - lo
                    kt = wpool.tile([NR, n, 2, P], U8, tag=f"k0_{lo}", name=f"k0_{lo}")
                    eng.dma_start(out=kt, in_=kdr_d[b, :, lo:hi])
                    vt = wpool.tile([P, n, D + 1], F16, tag=f"v0_{lo}", name=f"v0_{lo}")
                    eng.dma_start(out=vt, in_=v_r[b, :, lo:hi])
                    kviews.extend(kt[:, c] for c in range(n))
                    vviews.extend(vt[:, c] for c in range(n))

                q_load(1)
                kv_load(4, 8, eng=nc.gpsimd)
                q_load(2)
                kv_load(8, 12)
                q_load(3)
                kv_load(12, 16)
            else:
                qdr = wpool.tile([NR, NQT, 2, QW], U8, tag="qdr", name=f"qdr{b}")
                nc.sync.dma_start(out=qdr, in_=qdr_d[b])
                qdrs = [qdr[:, i] for i in range(NQT)]
                kdr = wpool.tile([NR, NCH, 2, P], U8, tag="kdr", name=f"kdr{b}")
                nc.sync.dma_start(out=kdr, in_=kdr_d[b])
                vv = wpool.tile([P, NCH, D + 1], F16, tag="vv", name=f"vv{b}")
                nc.sync.dma_start(out=vv, in_=v_r[b])
                kviews = [kdr[:, c] for c in range(NCH)]
                vviews = [vv[:, c] for c in range(NCH)]
            return qdrs, kviews, vviews

        pvq = []  # global PV pipeline: next tile's S^T slots in
                  # before this tile's last PVs, smoothing tile boundaries

        def emit_pv(e):
            u, i, pexp, otp, vviews, start, stop, fin = e
            if u == 2 * i + 1:
                nc.tensor.matmul(
                    out=otp[0 : D + 1, 256:QW],
                    lhsT=vviews[2 * u],
                    rhs=pexp[:, 0:256],
                    start=start, stop=False,
                )
                nc.tensor.matmul(
                    out=otp[0 : D + 1, 384:QW],
                    lhsT=vviews[2 * u + 1],
                    rhs=pexp[:, 256:384],
                    start=False, stop=stop,
                )
            elif u == 2 * i:
                nc.tensor.matmul(
                    out=otp[0 : D + 1, :],
                    lhsT=vviews[2 * u],
                    rhs=pexp[:, 0:QW],
                    start=start, stop=False,
                )
                nc.tensor.matmul(
                    out=otp[0 : D + 1, P:QW],
                    lhsT=vviews[2 * u + 1],
                    rhs=pexp[:, QW : QW + 384],
                    start=False, stop=stop,
                )
            else:
                for h in range(2):
                    nc.tensor.matmul(
                        out=otp[0 : D + 1, :],
                        lhsT=vviews[2 * u + h],
                        rhs=pexp[:, h * QW : (h + 1) * QW],
                        start=start and h == 0,
                        stop=stop and h == 1,
                    )
            if fin is not None:
                b, i2, otp2, last_tile = fin
                osb = osb_pool.tile(
                    [D + 1, QW], F16, tag="osb", name=f"osb{b}_{i2}"
                )
                if last_tile:
                    for hh in range(2):
                        sl = slice(hh * 256, (hh + 1) * 256)
                        nc.scalar.copy(
                            out=osb[:, sl], in_=otp2[0 : D + 1, sl]
                        )
                        nc.scalar.dma_start(out=o_d[b, i2, :, sl], in_=osb[:, sl])
                else:
                    nc.vector.tensor_copy(out=osb, in_=otp2[0 : D + 1, :])
                    nc.gpsimd.dma_start(out=o_d[b, i2], in_=osb)

        def compute_qtile(b, i, qdrs, kviews, vviews, last_tile):
            qdr8 = qdrs[i].bitcast(F8E4)
            otp = ot_ps.tile([P, QW], F32, tag="ot", name=f"ot{b}_{i}")
            if i == 0:
                order = [0, 1]
            else:
                order = [0, 1, 2 * i, 2 * i + 1] + list(range(2, 2 * i))
            last_u = order[-1]

            for oidx, u in enumerate(order):
                start = oidx == 0
                stop = u == last_u
                # odd non-diagonal pairs on the DVE custom op, the rest
                # (incl. the small diagonal pair) on ACT - balances the
                # measured engine busy times
                is_dve = (u % 2 == 1) and (u != 2 * i + 1)
                if last_tile:
                    # final tile: split its two pairs across engines so the
                    # closing exp chain runs on ACT and DVE in parallel
                    is_dve = u == 0
                stp = st_ps.tile([P, 2 * QW], F32, tag="st", name=f"st{b}_{i}_{u}")
                pexp = pepool.tile([P, 2 * QW], F16, tag="pe", name=f"pe{b}_{i}_{u}")

                if u == 2 * i + 1:
                    # diagonal pair, queries 256:512; chunk B only covers
                    # queries 384:512 (the rest is fully masked)
                    nc.tensor.matmul(
                        out=stp[:, 0:256],
                        lhsT=kviews[2 * u].bitcast(F8E4),
                        rhs=qdr8[:, :, 256:QW],
                        perf_mode=DR, start=True, stop=True,
                    )
                    nc.tensor.matmul(
                        out=stp[:, 256:384],
                        lhsT=kviews[2 * u + 1].bitcast(F8E4),
                        rhs=qdr8[:, :, 384:QW],
                        perf_mode=DR, start=True, stop=True,
                    )
                    width = 384
                elif u == 2 * i:
                    # leading masked pair; chunk B only covers queries 128:512
                    nc.tensor.matmul(
                        out=stp[:, 0:QW],
                        lhsT=kviews[2 * u].bitcast(F8E4),
                        rhs=qdr8[:, :, :],
                        perf_mode=DR, start=True, stop=True,
                    )
                    nc.tensor.matmul(
                        out=stp[:, QW : QW + 384],
                        lhsT=kviews[2 * u + 1].bitcast(F8E4),
                        rhs=qdr8[:, :, 128:QW],
                        perf_mode=DR, start=True, stop=True,
                    )
                    width = QW + 384
                else:
                    for h in range(2):
                        nc.tensor.matmul(
                            out=stp[:, h * QW : (h + 1) * QW],
                            lhsT=kviews[2 * u + h].bitcast(F8E4),
                            rhs=qdr8[:, :, :],
                            perf_mode=DR, start=True, stop=True,
                        )
                    width = 2 * QW

                if is_dve:
                    nc.vector._custom_dve(
                        exp2,
                        out=pexp[:, 0:width].bitcast(U16),
                        in0=stp[:, 0:width],
                        in1=c3t,
                        s0=MAGIC,
                        s1=CCONST,
                        imm2=BCONST,
                    )
                else:
                    nc.scalar.activation(
                        out=pexp[:, 0:width],
                        in_=stp[:, 0:width],
                        func=mybir.ActivationFunctionType.Exp,
                        scale=ACT_SCALE,
                        bias=abias[:, 0:1],
                    )
                if u == 2 * i + 1:
                    nc.vector.tensor_mul(out=pexp[:, 0:P], in0=pexp[:, 0:P], in1=tri0)
                    nc.vector.tensor_mul(
                        out=pexp[:, 256:384], in0=pexp[:, 256:384], in1=tri0
                    )
                elif u == 2 * i:
                    nc.vector.tensor_mul(out=pexp[:, 0:P], in0=pexp[:, 0:P], in1=tri0)
                    nc.vector.tensor_mul(
                        out=pexp[:, QW : QW + P], in0=pexp[:, QW : QW + P], in1=tri0
                    )
                fin = (b, i, otp, last_tile) if stop else None
                pvq.append((u, i, pexp, otp, vviews, start, stop, fin))
                if len(pvq) > 4:
                    emit_pv(pvq.pop(0))

        batches = [load_batch(b) for b in range(BL)]
        tiles = []
        for b in range(BL):
            order_i = range(NQT - 1, -1, -1) if b == BL - 1 else range(NQT)
            for i in order_i:
                tiles.append((b, i))
        for n, (b, i) in enumerate(tiles):
            compute_qtile(b, i, *batches[b], last_tile=(n == len(tiles) - 1))
        for e in pvq:
            emit_pv(e)

    return nc


_NC_CACHE = None


def _get_nc():
    global _NC_CACHE
    if _NC_CACHE is None:
        nc = build_nc()
        nc.finalize()
        _NC_CACHE = nc
    return _NC_CACHE


def prep_inputs(queries, keys, values):
    """Host-side shard + layout prep (numpy only)."""
    E4NP = ml_dtypes.float8_e4m3
    q = np.asarray(queries, dtype=np.float32)
    k = np.asarray(keys, dtype=np.float32)
    v = np.asarray(values, dtype=np.float32)
    assert q.shape == (B, T, D), q.shape

    qT = (q * SQK).transpose(0, 2, 1)                # [B, 64, T] fp32
    kT = (k * SQK).transpose(0, 2, 1)
    qh = qT.astype(E4NP)
    ql = (qT - qh.astype(np.float32)).astype(E4NP)
    kh = kT.astype(E4NP)
    kl = (kT - kh.astype(np.float32)).astype(E4NP)

    # contraction row table: row r lives at (partition r % 128, ktile r // 128)
    #   rows   0..63  : qh[d] * kh[d]
    #   rows  64..127 : ql[d] * kh[d]
    #   rows 128..191 : qh[d] * kl[d]
    #   row  192      : 128 * 112  = 14336
    #   row  193      : 128 * 4    =   512   (sum = XBIAS = 14848)
    #   rows 194..255 : zero pad
    qdr = np.zeros((B, NR, 2, T), dtype=E4NP)
    kdr = np.zeros((B, NR, 2, T), dtype=E4NP)

    def put(r, qrow, krow):
        qdr[:, r % NR, r // NR, :] = qrow
        kdr[:, r % NR, r // NR, :] = krow

    for d in range(D):
        put(d, qh[:, d, :], kh[:, d, :])
        put(D + d, ql[:, d, :], kh[:, d, :])
        put(2 * D + d, qh[:, d, :], kl[:, d, :])
    ones = np.ones((B, T), dtype=E4NP)
    put(3 * D, ones * E4NP(128.0), ones * E4NP(112.0))
    put(3 * D + 1, ones * E4NP(128.0), ones * E4NP(4.0))

    va = np.concatenate(
        [v.astype(np.float16), np.ones((B, T, 1), np.float16)], axis=-1
    )
    # [B, NR, 2, T] -> [B, NR, NQT, 2, QW] (q) / [B, NR, NCH, 2, P] (k):
    # per-matmul blocks contiguous in SBUF
    qdr_t = qdr.reshape(B, NR, 2, NQT, QW).transpose(0, 1, 3, 2, 4)
    kdr_t = kdr.reshape(B, NR, 2, NCH, P).transpose(0, 1, 3, 2, 4)
    arrs = dict(
        qdr=np.ascontiguousarray(qdr_t).view(np.uint8),
        kdr=np.ascontiguousarray(kdr_t).view(np.uint8),
        v=np.ascontiguousarray(va),
    )
    return [
        {k_: a[c * BL : (c + 1) * BL] for k_, a in arrs.items()}
        for c in range(NCORES)
    ]


def run(queries, keys, values, trace=False):
    nc = _get_nc()
    core_ids = list(range(NCORES))
    in_maps = prep_inputs(queries, keys, values)
    try:
        res = run_bass_kernel_spmd(nc, in_maps, core_ids, trace=trace)
    except Exception:
        res = run_bass_kernel_spmd(nc, in_maps, core_ids, trace=trace)
    outs = []
    for c in core_ids:
        ot = res.results[c]["o"].astype(np.float32)   # [BL, NQT, D+1, QW]
        o = ot[:, :, :D, :] / ot[:, :, D : D + 1, :]  # divide by sums
        # [BL, NQT, D, QW] -> [BL, NQT, QW, D] -> [BL, T, D]
        outs.append(o.transpose(0, 1, 3, 2).reshape(BL, T, D))
    return np.concatenate(outs, axis=0), res


def kernel(queries, keys, values):
    out, _ = run(queries, keys, values, trace=False)
    return out


# revision 28
# speedup vs baseline: 1.0132x; 1.0132x over previous
"""Causal dot-product attention for Trainium2 (Bass/Tile), 8-core SPMD. v3.

Problem: B=32, T=2048, D=64 fp32.  reference:
    O = softmax(mask(Q K^T / sqrt(D))) V      (causal mask, per batch)

Sharding: pure batch parallelism - 4 batches per NeuronCore, no collectives.
~82.4 us HW (from the 94.3 us v2 baseline).

v3 design (vs v2):
  * S^T matmuls use fp8e4 + DoubleRow with error-compensated operands:
    the 2x128-row contraction budget holds qh*kh + ql*kh + qh*kl (3
    terms x 64 d-rows) plus two bias rows (128*112 + 128*4 = 14848,
    each factor e4m3-exact) = 194 rows in [128 partitions, 2 k-tiles]
    (pad to 128 partitions - smaller matmuls wedge the device).  Score
    error ~0.003; 2.3e-3 end-to-end rel err on HW.
    NOTE: on HW a DoubleRow matmul still streams 1 out-col/cycle (the
    cost model's 0.5 does not materialize; byte-interleaved ifmap is
    WORSE), so fp8-DR only buys contraction depth, not rate.  The PE
    floor here is (causal column count) x 1 cyc = ~58 us/core; this
    kernel runs the PE at ~97% occupancy against that floor.
  * All pairs carry the +14848 bias; the ACT exp path compensates via
    the activation bias argument (-14848*ln2/1024).
  * Fully-masked 128-col blocks are trimmed from S^T/exp/PV (chunk B of
    the leading pair covers queries 128:512; of the diagonal pair,
    384:512) - all masks reduce to the single tri0 triangle.
  * exp split: odd non-diagonal pairs on the DVE custom fp16-bits op,
    the rest on ACT (measured-balanced); masks + epilogue casts on DVE.
  * PV stays fp16 (fp8 pexp fails the error budget: e4m3 lacks range,
    e5m2 lacks mantissa - 0.10-0.14 rel err in simulation).
  * Global cross-tile PV software pipeline (depth 2): the next tile's
    S^T fills the PE while the previous tile's last exps drain.
  * Head: batch-0's first blocks ride the Activation HWDGE queue ahead
    of the sync preamble; remaining b0 loads are split/ordered by
    consumption.  Tail: non-final output DMAs ride the Pool SW-DGE
    (prompt triggers), the final tile's copies+DMAs ride ACT, and the
    final tile's two exps run on different engines.
"""

import os

os.environ.setdefault("NEURON_RT_RESET_CORES", "1")

import numpy as np
import ml_dtypes

import concourse.bacc as bacc
import concourse.mybir as mybir
import concourse.tile as tile
from concourse.bass_utils import run_bass_kernel_spmd

B, T, D = 32, 2048, 64
NCORES = 8
BL = B // NCORES            # batches per core
P = 128                     # partitions / key-chunk size
NCH = T // P                # key chunks per batch (16)
QW = 512                    # query-tile width
NQT = T // QW               # query tiles per batch (4)

NR = 128                    # contraction partitions for the fp8 DR matmul
                            # (194 live rows + 62 zero pad; <128-partition
                            # matmuls are known-problematic on HW)
LOG2E = float(np.log2(np.e))
SQK = float(np.sqrt(128.0 * LOG2E))     # q/k prescale; SQK^2 = 128*log2e
XBIAS = 14848.0                         # (15-1)*1024 + 512
ACT_SCALE = float(np.log(2.0) / 1024.0) # exp(x*ln2/1024) = 2^(x/1024)
ACT_BIAS = float(-XBIAS * np.log(2.0) / 1024.0)

# custom DVE exp2 constants (fp16 bits domain, 1024-grid)
MAGIC = float(1.5 * 2**33)
ACONST = 423.935717
BCONST = 0.995047887        # linear (per rs)
CCONST = 3.35938581e-4      # quadratic (per rs^2)

F32 = mybir.dt.float32
F16 = mybir.dt.float16
U16 = mybir.dt.uint16
U8 = mybir.dt.uint8
F8E4 = mybir.dt.float8e4
DR = mybir.MatmulPerfMode.DoubleRow

PREWARM_N = int(os.environ.get("ATTN_PREWARM_N", "12"))


def _register_exp2_op():
    """Register the EXP2_FP16_BITS_ANT custom DVE op (idempotent)."""
    from concourse.dve_ops import (
        DveOp,
        OPS,
        _SUB_OPCODE_FOR_NAME,
        CUSTOM_DVE_SPECS,
    )
    from concourse.dve_spec import (
        Spec,
        Src0,
        C0,
        C1,
        C2,
        C3,
        lower,
        _spill_c3_to_src1,
        _has_src1,
    )
    from concourse.dve_uop import DveOpSpec

    name = "EXP2_FP16_BITS_ANT"
    if name in _SUB_OPCODE_FOR_NAME:
        return next(op for op in OPS if op.name == name)

    def _ref(in0, in1, s0, s1, imm2):
        X = in0.astype(np.float32)
        u = (X + np.float32(s0)).astype(np.float32)
        v = (u - np.float32(s0)).astype(np.float32)
        rs = (X - v).astype(np.float32)
        G = ((rs * np.float32(s1) + np.float32(imm2)) * rs).astype(np.float32)
        c3 = np.asarray(in1, np.float32).reshape(-1, 1)
        return ((G + v) + c3).astype(np.float32)

    u = Src0 + C0
    v = u - C0
    rs = Src0 - v
    G = (rs * C1 + C2) * rs
    body = _spill_c3_to_src1((G + v) + C3)
    spec = Spec(body=body, reference=_ref)
    row = max(_SUB_OPCODE_FOR_NAME.values()) + 1
    _SUB_OPCODE_FOR_NAME[name] = row
    uops = lower(spec, ver="v3")
    sha = DveOpSpec(name=name, opcode=row, uops=uops, rd1_en=_has_src1(spec)).sha(
        "v3"
    )
    op = DveOp(name, spec, subdim=False, uops_sha={"v3": sha})
    OPS.append(op)
    CUSTOM_DVE_SPECS[name] = spec
    return op


def build_nc():
    from contextlib import ExitStack

    exp2 = _register_exp2_op()
    nc = bacc.Bacc()
    # host-prepped inputs:
    #   qdr: fp8e4 bytes [NR, NQT, 2, QW] per batch - DoubleRow moving
    #        operand; row (p, j) holds the q-side factor of contraction row
    #        j*128+p (see prep_inputs); per-qtile blocks are contiguous
    #   kdr: matching k-side stationary operand [NR, NCH, 2, P]
    #   v:   V with ones column, fp16 [T, 65]
    qdr_d = nc.dram_tensor("qdr", [BL, NR, NQT, 2, QW], U8, kind="ExternalInput")
    kdr_d = nc.dram_tensor("kdr", [BL, NR, NCH, 2, P], U8, kind="ExternalInput")
    v_d = nc.dram_tensor("v", [BL, T, D + 1], F16, kind="ExternalInput")
    o_d = nc.dram_tensor("o", [BL, NQT, D + 1, QW], F16, kind="ExternalOutput")

    with tile.TileContext(nc) as tc, ExitStack() as ctx:
        singles = ctx.enter_context(tc.tile_pool(name="singles", bufs=1))
        wpool = ctx.enter_context(tc.tile_pool(name="wts", bufs=4))
        pepool = ctx.enter_context(tc.tile_pool(name="pexp", bufs=8))
        osb_pool = ctx.enter_context(tc.tile_pool(name="osb", bufs=3))
        st_ps = ctx.enter_context(tc.tile_pool(name="stps", bufs=3, space="PSUM"))
        ot_ps = ctx.enter_context(tc.tile_pool(name="otps", bufs=2, space="PSUM"))

        # critical first input blocks on the Activation HWDGE queue: ACT's
        # sequencer reaches these long before the sync engine finishes its
        # preamble, so batch 0 tile 0's data lands several us earlier.
        q0t0 = wpool.tile([NR, 2, QW], U8, tag="q0_0", name="q0_0")
        nc.scalar.dma_start(out=q0t0, in_=qdr_d[0, :, 0])
        v_r = v_d.rearrange("b (c p) d -> b p c d", p=P)
        k0t0 = wpool.tile([NR, 4, 2, P], U8, tag="k0_0", name="k0_0")
        nc.scalar.dma_start(out=k0t0, in_=kdr_d[0, :, 0:4])
        v0t0 = wpool.tile([P, 4, D + 1], F16, tag="v0_0", name="v0_0")
        nc.scalar.dma_start(out=v0t0, in_=v_r[0, :, 0:4])

        if PREWARM_N:
            # fp16 matmul burst on dummy data during the initial input-DMA
            # stall: opens the HAM clock gate / pstate before the real work.
            wsrc = singles.tile([P, QW], F16)
            nc.gpsimd.memset(wsrc, 0.5)
            wps = ot_ps.tile([P, QW], F32, tag="ot", name="warm")
            for _ in range(PREWARM_N):
                nc.tensor.matmul(
                    out=wps, lhsT=wsrc[:, 0:P], rhs=wsrc, start=True, stop=True
                )

        # precomputed 0/1 causal masks (fp16), applied by DVE multiplies:
        #   tri0: keep where f >= p      (the diagonal 128-triangle)
        #   msk1: keep where f >= 128+p  (one full masked chunk + triangle)
        tri0f = singles.tile([P, P], F32)
        nc.vector.memset(tri0f, 1.0)
        nc.gpsimd.affine_select(
            out=tri0f, in_=tri0f, compare_op=mybir.AluOpType.is_ge, fill=0.0,
            base=0, channel_multiplier=-1, pattern=[[1, P]],
        )
        tri0 = singles.tile([P, P], F16)
        nc.vector.tensor_copy(out=tri0, in_=tri0f)
        c3t = singles.tile([P, 1], F32)
        nc.vector.memset(c3t, ACONST)
        abias = singles.tile([P, 1], F32)
        nc.vector.memset(abias, ACT_BIAS)

        # preload the exp table set during the head DMA stall
        dummy = singles.tile([P, 8], F32)
        nc.vector.memset(dummy, 0.0)
        dummo = singles.tile([P, 8], F16)
        nc.scalar.activation(
            out=dummo, in_=dummy, func=mybir.ActivationFunctionType.Exp,
            scale=ACT_SCALE,
        )

        def load_batch(b):
            if b == 0:
                # consumption order: tile1 needs qdr1 + chunks 4..7 before
                # tile2's qdr2 + chunks 8..15
                qdrs = [q0t0]
                kviews = [k0t0[:, c] for c in range(4)]
                vviews = [v0t0[:, c] for c in range(4)]

                def q_load(i):
                    qt = wpool.tile([NR, 2, QW], U8, tag=f"q0_{i}", name=f"q0_{i}")
                    nc.sync.dma_start(out=qt, in_=qdr_d[b, :, i])
                    qdrs.append(qt)

                def kv_load(lo, hi, eng=None):
                    eng = eng or nc.sync
                    n = hi - lo
                    kt = wpool.tile([NR, n, 2, P], U8, tag=f"k0_{lo}", name=f"k0_{lo}")
                    eng.dma_start(out=kt, in_=kdr_d[b, :, lo:hi])
                    vt = wpool.tile([P, n, D + 1], F16, tag=f"v0_{lo}", name=f"v0_{lo}")
                    eng.dma_start(out=vt, in_=v_r[b, :, lo:hi])
                    kviews.extend(kt[:, c] for c in range(n))
                    vviews.extend(vt[:, c] for c in range(n))

                q_load(1)
                kv_load(4, 8)
                q_load(2)
                kv_load(8, 12)
                q_load(3)
                kv_load(12, 16)
            else:
                qdr = wpool.tile([NR, NQT, 2, QW], U8, tag="qdr", name=f"qdr{b}")
                nc.sync.dma_start(out=qdr, in_=qdr_d[b])
                qdrs = [qdr[:, i] for i in range(NQT)]
                kdr = wpool.tile([NR, NCH, 2, P], U8, tag="kdr", name=f"kdr{b}")
                nc.sync.dma_start(out=kdr, in_=kdr_d[b])
                vv = wpool.tile([P, NCH, D + 1], F16, tag="vv", name=f"vv{b}")
                nc.sync.dma_start(out=vv, in_=v_r[b])
                kviews = [kdr[:, c] for c in range(NCH)]
                vviews = [vv[:, c] for c in range(NCH)]
            return qdrs, kviews, vviews

        pvq = []  # global PV pipeline: next tile's S^T slots in
                  # before this tile's last PVs, smoothing tile boundaries

        def emit_pv(e):
            u, i, pexp, otp, vviews, start, stop, fin = e
            if u == 2 * i + 1:
                nc.tensor.matmul(
                    out=otp[0 : D + 1, 256:QW],
                    lhsT=vviews[2 * u],
                    rhs=pexp[:, 0:256],
                    start=start, stop=False,
                )
                nc.tensor.matmul(
                    out=otp[0 : D + 1, 384:QW],
                    lhsT=vviews[2 * u + 1],
                    rhs=pexp[:, 256:384],
                    start=False, stop=stop,
                )
            elif u == 2 * i:
                nc.tensor.matmul(
                    out=otp[0 : D + 1, :],
                    lhsT=vviews[2 * u],
                    rhs=pexp[:, 0:QW],
                    start=start, stop=False,
                )
                nc.tensor.matmul(
                    out=otp[0 : D + 1, P:QW],
                    lhsT=vviews[2 * u + 1],
                    rhs=pexp[:, QW : QW + 384],
                    start=False, stop=stop,
                )
            else:
                for h in range(2):
                    nc.tensor.matmul(
                        out=otp[0 : D + 1, :],
                        lhsT=vviews[2 * u + h],
                        rhs=pexp[:, h * QW : (h + 1) * QW],
                        start=start and h == 0,
                        stop=stop and h == 1,
                    )
            if fin is not None:
                b, i2, otp2, last_tile = fin
                osb = osb_pool.tile(
                    [D + 1, QW], F16, tag="osb", name=f"osb{b}_{i2}"
                )
                if last_tile:
                    for hh in range(2):
                        sl = slice(hh * 256, (hh + 1) * 256)
                        nc.scalar.copy(
                            out=osb[:, sl], in_=otp2[0 : D + 1, sl]
                        )
                        nc.scalar.dma_start(out=o_d[b, i2, :, sl], in_=osb[:, sl])
                else:
                    nc.vector.tensor_copy(out=osb, in_=otp2[0 : D + 1, :])
                    nc.gpsimd.dma_start(out=o_d[b, i2], in_=osb)

        def compute_qtile(b, i, qdrs, kviews, vviews, last_tile):
            qdr8 = qdrs[i].bitcast(F8E4)
            otp = ot_ps.tile([P, QW], F32, tag="ot", name=f"ot{b}_{i}")
            if i == 0:
                order = [0, 1]
            else:
                order = [0, 1, 2 * i, 2 * i + 1] + list(range(2, 2 * i))
            last_u = order[-1]

            for oidx, u in enumerate(order):
                start = oidx == 0
                stop = u == last_u
                # odd non-diagonal pairs on the DVE custom op, the rest
                # (incl. the small diagonal pair) on ACT - balances the
                # measured engine busy times
                is_dve = (u % 2 == 1) and (u != 2 * i + 1)
                if last_tile:
                    # final tile: split its two pairs across engines so the
                    # closing exp chain runs on ACT and DVE in parallel
                    is_dve = u == 0
                stp = st_ps.tile([P, 2 * QW], F32, tag="st", name=f"st{b}_{i}_{u}")
                pexp = pepool.tile([P, 2 * QW], F16, tag="pe", name=f"pe{b}_{i}_{u}")

                if u == 2 * i + 1:
                    # diagonal pair, queries 256:512; chunk B only covers
                    # queries 384:512 (the rest is fully masked)
                    nc.tensor.matmul(
                        out=stp[:, 0:256],
                        lhsT=kviews[2 * u].bitcast(F8E4),
                        rhs=qdr8[:, :, 256:QW],
                        perf_mode=DR, start=True, stop=True,
                    )
                    nc.tensor.matmul(
                        out=stp[:, 256:384],
                        lhsT=kviews[2 * u + 1].bitcast(F8E4),
                        rhs=qdr8[:, :, 384:QW],
                        perf_mode=DR, start=True, stop=True,
                    )
                    width = 384
                elif u == 2 * i:
                    # leading masked pair; chunk B only covers queries 128:512
                    nc.tensor.matmul(
                        out=stp[:, 0:QW],
                        lhsT=kviews[2 * u].bitcast(F8E4),
                        rhs=qdr8[:, :, :],
                        perf_mode=DR, start=True, stop=True,
                    )
                    nc.tensor.matmul(
                        out=stp[:, QW : QW + 384],
                        lhsT=kviews[2 * u + 1].bitcast(F8E4),
                        rhs=qdr8[:, :, 128:QW],
                        perf_mode=DR, start=True, stop=True,
                    )
                    width = QW + 384
                else:
                    for h in range(2):
                        nc.tensor.matmul(
                            out=stp[:, h * QW : (h + 1) * QW],
                            lhsT=kviews[2 * u + h].bitcast(F8E4),
                            rhs=qdr8[:, :, :],
                            perf_mode=DR, start=True, stop=True,
                        )
                    width = 2 * QW

                if is_dve:
                    nc.vector._custom_dve(
                        exp2,
                        out=pexp[:, 0:width].bitcast(U16),
                        in0=stp[:, 0:width],
                        in1=c3t,
                        s0=MAGIC,
                        s1=CCONST,
                        imm2=BCONST,
                    )
                else:
                    nc.scalar.activation(
                        out=pexp[:, 0:width],
                        in_=stp[:, 0:width],
                        func=mybir.ActivationFunctionType.Exp,
                        scale=ACT_SCALE,
                        bias=abias[:, 0:1],
                    )
                if u == 2 * i + 1:
                    nc.vector.tensor_mul(out=pexp[:, 0:P], in0=pexp[:, 0:P], in1=tri0)
                    nc.vector.tensor_mul(
                        out=pexp[:, 256:384], in0=pexp[:, 256:384], in1=tri0
                    )
                elif u == 2 * i:
                    nc.vector.tensor_mul(out=pexp[:, 0:P], in0=pexp[:, 0:P], in1=tri0)
                    nc.vector.tensor_mul(
                        out=pexp[:, QW : QW + P], in0=pexp[:, QW : QW + P], in1=tri0
                    )
                fin = (b, i, otp, last_tile) if stop else None
                pvq.append((u, i, pexp, otp, vviews, start, stop, fin))
                if len(pvq) > 4:
                    emit_pv(pvq.pop(0))

        batches = [load_batch(b) for b in range(BL)]
        tiles = []
        for b in range(BL):
            order_i = range(NQT - 1, -1, -1) if b == BL - 1 else range(NQT)
            for i in order_i:
                tiles.append((b, i))
        for n, (b, i) in enumerate(tiles):
            compute_qtile(b, i, *batches[b], last_tile=(n == len(tiles) - 1))
        for e in pvq:
            emit_pv(e)

    return nc


_NC_CACHE = None


def _get_nc():
    global _NC_CACHE
    if _NC_CACHE is None:
        nc = build_nc()
        nc.finalize()
        _NC_CACHE = nc
    return _NC_CACHE


def prep_inputs(queries, keys, values):
    """Host-side shard + layout prep (numpy only)."""
    E4NP = ml_dtypes.float8_e4m3
    q = np.asarray(queries, dtype=np.float32)
    k = np.asarray(keys, dtype=np.float32)
    v = np.asarray(values, dtype=np.float32)
    assert q.shape == (B, T, D), q.shape

    qT = (q * SQK).transpose(0, 2, 1)                # [B, 64, T] fp32
    kT = (k * SQK).transpose(0, 2, 1)
    qh = qT.astype(E4NP)
    ql = (qT - qh.astype(np.float32)).astype(E4NP)
    kh = kT.astype(E4NP)
    kl = (kT - kh.astype(np.float32)).astype(E4NP)

    # contraction row table: row r lives at (partition r % 128, ktile r // 128)
    #   rows   0..63  : qh[d] * kh[d]
    #   rows  64..127 : ql[d] * kh[d]
    #   rows 128..191 : qh[d] * kl[d]
    #   row  192      : 128 * 112  = 14336
    #   row  193      : 128 * 4    =   512   (sum = XBIAS = 14848)
    #   rows 194..255 : zero pad
    qdr = np.zeros((B, NR, 2, T), dtype=E4NP)
    kdr = np.zeros((B, NR, 2, T), dtype=E4NP)

    def put(r, qrow, krow):
        qdr[:, r % NR, r // NR, :] = qrow
        kdr[:, r % NR, r // NR, :] = krow

    for d in range(D):
        put(d, qh[:, d, :], kh[:, d, :])
        put(D + d, ql[:, d, :], kh[:, d, :])
        put(2 * D + d, qh[:, d, :], kl[:, d, :])
    ones = np.ones((B, T), dtype=E4NP)
    put(3 * D, ones * E4NP(128.0), ones * E4NP(112.0))
    put(3 * D + 1, ones * E4NP(128.0), ones * E4NP(4.0))

    va = np.concatenate(
        [v.astype(np.float16), np.ones((B, T, 1), np.float16)], axis=-1
    )
    # [B, NR, 2, T] -> [B, NR, NQT, 2, QW] (q) / [B, NR, NCH, 2, P] (k):
    # per-matmul blocks contiguous in SBUF
    qdr_t = qdr.reshape(B, NR, 2, NQT, QW).transpose(0, 1, 3, 2, 4)
    kdr_t = kdr.reshape(B, NR, 2, NCH, P).transpose(0, 1, 3, 2, 4)
    arrs = dict(
        qdr=np.ascontiguousarray(qdr_t).view(np.uint8),
        kdr=np.ascontiguousarray(kdr_t).view(np.uint8),
        v=np.ascontiguousarray(va),
    )
    return [
        {k_: a[c * BL : (c + 1) * BL] for k_, a in arrs.items()}
        for c in range(NCORES)
    ]


def run(queries, keys, values, trace=False):
    nc = _get_nc()
    core_ids = list(range(NCORES))
    in_maps = prep_inputs(queries, keys, values)
    try:
        res = run_bass_kernel_spmd(nc, in_maps, core_ids, trace=trace)
    except Exception:
        res = run_bass_kernel_spmd(nc, in_maps, core_ids, trace=trace)
    outs = []
    for c in core_ids:
        ot = res.results[c]["o"].astype(np.float32)   # [BL, NQT, D+1, QW]
        o = ot[:, :, :D, :] / ot[:, :, D : D + 1, :]  # divide by sums
        # [BL, NQT, D, QW] -> [BL, NQT, QW, D] -> [BL, T, D]
        outs.append(o.transpose(0, 1, 3, 2).reshape(BL, T, D))
    return np.concatenate(outs, axis=0), res


def kernel(queries, keys, values):
    out, _ = run(queries, keys, values, trace=False)
    return out
